# revision 6
# baseline (speedup 1.0000x reference)
"""Multi-head self-attention (CMHSAttn) Trainium2 kernel — hybrid-exp v2.

Problem: x (1, 128, 64, 64) fp32, W_qkv (384, 128) fp32.
  qkv = 1x1-conv(x, W_qkv); per head h (8 heads, d_head=16):
  q,k,v from qkv channels [48h:48h+16], [48h+16:48h+32], [48h+32:48h+48];
  out = softmax(q k^T / sqrt(128)) v, laid out channel-major (128, 64, 64).

Sharding: one head per NeuronCore (8 cores), pure data parallel, no
collectives. Each core receives the full x (bf16, channel-major (128, 4096))
plus its head's weight slices, and computes its 16 output channels.

v2 changes vs the 147us baseline (which was ACT-bound on softmax exp):
  - Hybrid exp: the scalar engine (ACT, exact spline exp @1.2G/lane) and the
    vector engine (DVE) split the 16.8M-element exp work per core. DVE
    computes a Schraudolph-style approximate exp in ONE tensor_scalar
    instruction: i16 = rint(A*s + B) bit-viewed as bf16 (exp/mantissa trick,
    +-4% per-element, mean-centered; softmax averaging keeps end-to-end
    rel err ~5e-3 vs the 2e-2 budget).
  - Normalization without nc.vector.reciprocal (6 cyc/elem): Schraudolph
    reciprocal seed (int-domain K - bits(d)) + one Newton step, 4 cheap DVE
    instructions per q-chunk, reading the PSUM accumulator directly.
  - Ascending k-group order so the first exp only needs the first kt chunk.

Per-core algorithm (matmuls bf16, fp32 PSUM):
  - QT/KT = W_{q,k} @ x, replicated at partition offsets 0/32/64 so score
    matmuls pack 3-wide into PE row groups (K=16 each).
  - V2 (128, 48*32): per 128-position chunk kj, cols [48kj:48kj+16] = V chunk
    (position-major), [+16:+32] zeros, [+32:+48] = 1.0 (softmax-denominator
    rows; zero padding keeps partition bases 32-aligned downstream).
  - For each q-chunk (512) and k-group (3 k-tiles of 128): S^T tiles
    (k-partition, q-free) via packed matmuls -> exp over (128, <=1536) PSUM
    on ACT or DVE per a fixed assignment pattern -> P bf16 -> matmuls
    accumulate O'' (48, 512) += V2_kj^T @ P_kj.
  - out^T = O''[0:16] * newton_recip(O''[32:48]); DMA to HBM.
"""

import math

import ml_dtypes
import numpy as np

D_MODEL = 128
N = 4096  # 64*64 positions
DH = 16  # head dim
NH = 8  # heads = cores
QC = 512  # q-chunk (one PSUM bank of fp32)
NQC = N // QC  # 8
KT = 128  # k positions per score tile
NKJ = N // KT  # 32
# k-tile groups: 3-wide (PE row groups 0/32/64) except the last
GROUPS = [(g * 3, min(3, NKJ - g * 3)) for g in range((NKJ + 2) // 3)]
SCALE = 1.0 / math.sqrt(D_MODEL)

# Schraudolph exp in the bf16 bit domain: i16 = rint(s * A_EXP + B_EXP),
# bitcast bf16.  A folds the softmax scale; B centers the sawtooth error.
A_EXP = float(np.float32(128.0 * np.log2(np.e) * SCALE))
B_EXP = float(np.float32(127 * 128 - 8.0))
# fp32 reciprocal bit-trick magic (int domain): y0 = bits^-1(K - bits(d))
K_RECIP = float(0x7EF311C3)

# groups (by index into the per-qc group list) whose exp runs on DVE
DVE_GIS = frozenset({1, 4, 6, 9})

_NC_CACHE = {}


def _build_nc(
    legalize=True,
    loop_reps=None,
    pb_bufs=6,
    dve_gis=DVE_GIS,
    col_tile_ov=True,
    explicit_ldw=True,
):
    """Build the per-core Bass program. loop_reps wraps the whole body in a
    device-side For loop (used only for timing measurements)."""
    import concourse.bass as bass
    import concourse.mybir as mybir
    from concourse.tile import TileContext

    fp32 = mybir.dt.float32
    bf16 = mybir.dt.bfloat16
    i16 = mybir.dt.int16
    i32 = mybir.dt.int32
    EXP = mybir.ActivationFunctionType.Exp
    MULT = mybir.AluOpType.mult
    ADD = mybir.AluOpType.add
    SUB = mybir.AluOpType.subtract

    nc = bass.Bass(name="cmhs_attn_head")
    xb = nc.dram_tensor("xb", [D_MODEL, N], bf16, kind="ExternalInput")
    wq = nc.dram_tensor("wq", [D_MODEL, 128], bf16, kind="ExternalInput")
    wk = nc.dram_tensor("wk", [D_MODEL, 128], bf16, kind="ExternalInput")
    wv = nc.dram_tensor("wv", [D_MODEL, DH], bf16, kind="ExternalInput")
    out = nc.dram_tensor("out", [DH, N], fp32, kind="ExternalOutput")

    with (
        TileContext(nc) as tc,
        tc.tile_pool(name="const", bufs=1) as cpool,
        tc.tile_pool(name="pwork", bufs=pb_bufs) as ppool,
        tc.tile_pool(name="small", bufs=3) as mpool,
        tc.tile_pool(name="ps", bufs=2, space="PSUM") as pspool,
        tc.tile_pool(name="po", bufs=2, space="PSUM") as popool,
    ):
        if True:
            # ---- persistent SBUF tensors ----
            xb_sb = cpool.tile([D_MODEL, N], bf16, name="xb_sb")
            wq_sb = cpool.tile([D_MODEL, 128], bf16, name="wq_sb")
            wk_sb = cpool.tile([D_MODEL, 128], bf16, name="wk_sb")
            wv_sb = cpool.tile([D_MODEL, DH], bf16, name="wv_sb")
            qt = cpool.tile([D_MODEL, N], bf16, name="qt")  # replicated q^T
            kt = cpool.tile([D_MODEL, N], bf16, name="kt")  # replicated k^T
            # per k-chunk 48 cols: V (0:16) | zeros (16:32) | ones (32:48)
            v2 = cpool.tile([D_MODEL, NKJ * 48], bf16, name="v2")

            v2_v = v2.rearrange("p (j t) -> p j t", t=48)

            def proj_qk_group(dst, w_sb, c0, cn):
                # project q or k (replicated at partitions 0-15/32-47/64-79)
                # for x-chunks c0..c0+cn
                pj = pspool.tile([D_MODEL, 3 * QC], fp32, name="pj", tag="s")
                for t in range(cn):
                    c = c0 + t
                    nc.tensor.matmul(
                        pj[:, t * QC : (t + 1) * QC],
                        lhsT=w_sb[:],
                        rhs=xb_sb[:, c * QC : (c + 1) * QC],
                        start=True,
                        stop=True,
                    )
                nc.vector.tensor_copy(
                    out=dst[:, c0 * QC : (c0 + cn) * QC],
                    in_=pj[:, : cn * QC],
                )

            def proj_v_range(vp, kj0, kj1):
                # V chunks kj0..kj1 position-major via x-chunk-stationary MMs
                vp_v = vp.rearrange("p (j t) -> p j t", t=DH)
                for kj in range(kj0, kj1):
                    nc.tensor.matmul(
                        vp[:, kj * DH : (kj + 1) * DH],
                        lhsT=xb_sb[:, kj * KT : (kj + 1) * KT],
                        rhs=wv_sb[:],
                        start=True,
                        stop=True,
                    )
                nc.vector.tensor_copy(
                    out=v2_v[:, kj0:kj1, 0:DH],
                    in_=vp_v[:, kj0:kj1, :],
                )

            def score_exp_group(qc, c0, cn, use_dve):
                # packed score matmuls + one exp over the group's PSUM span
                qs = qc * QC
                sps = pspool.tile([D_MODEL, 3 * QC], fp32, name="sps", tag="s")
                for t in range(cn):
                    kj = c0 + t
                    ro = 32 * t  # PE row group offset
                    w_ap = kt[ro : ro + DH, kj * KT : (kj + 1) * KT]
                    if explicit_ldw:
                        nc.tensor.ldweights(w_ap, tile_position=(ro, 0))
                    nc.tensor.matmul(
                        sps[:, t * QC : (t + 1) * QC],
                        lhsT=w_ap,
                        rhs=qt[ro : ro + DH, qs : qs + QC],
                        start=True,
                        stop=True,
                    )
                pb = ppool.tile([D_MODEL, 3 * QC], bf16, name="pb", tag="p")
                if use_dve:
                    nc.vector.tensor_scalar(
                        out=pb[:, : cn * QC].bitcast(i16),
                        in0=sps[:, : cn * QC],
                        scalar1=A_EXP,
                        scalar2=B_EXP,
                        op0=MULT,
                        op1=ADD,
                    )
                else:
                    nc.scalar.activation(
                        pb[:, : cn * QC], sps[:, : cn * QC], EXP, scale=SCALE
                    )
                return pb

            def ov_group(o2, ov_state, pb, c0, cn):
                # col-tiled: even kj accumulate into o2[0:48] (PE col strips
                # 0-1), odd kj into o2[64:112] (strips 2-3) — the two streams
                # run concurrently on the PE; halves are summed in normalize.
                for t in range(cn):
                    kj = c0 + t
                    if col_tile_ov:
                        half = kj % 2
                        po, co = (0, 0) if half == 0 else (64, 64)
                        n_prev = ov_state[half]
                        ov_state[half] += 1
                        dst = o2[po : po + 48, :]
                        first = n_prev == 0
                        last = n_prev == NKJ // 2 - 1
                        tp = (0, co)
                    else:
                        n_prev = ov_state[0]
                        ov_state[0] += 1
                        dst = o2[0:48, :]
                        first = n_prev == 0
                        last = n_prev == NKJ - 1
                        tp = (0, 0)
                    w_ap = v2[:, kj * 48 : kj * 48 + 48]
                    if explicit_ldw:
                        nc.tensor.ldweights(w_ap, tile_position=tp)
                    nc.tensor.matmul(
                        dst,
                        lhsT=w_ap,
                        rhs=pb[:, t * QC : (t + 1) * QC],
                        start=first,
                        stop=last,
                        skip_group_check=True,
                        tile_position=tp,
                    )

            def normalize_and_store(qc, o2):
                # denominator rows hold d = sum_k exp (fp32); 1/d via
                # Schraudolph seed + one Newton step, sign-folded:
                #   y0 = bits^-1(K - bits(d)); z1 = (d*y0-2)*y0 = -y1
                #   ob = -(numer) * z1 = numer * y1
                if col_tile_ov:
                    c_ob = mpool.tile([48, QC], fp32, name="c_ob", tag="cob")
                    nc.vector.tensor_copy(out=c_ob[:], in_=o2[64:112, :])
                    nneg = mpool.tile([DH, QC], fp32, name="nneg", tag="nn")
                    nc.vector.scalar_tensor_tensor(
                        out=nneg[:], in0=o2[0:DH, :], scalar=-1.0,
                        in1=c_ob[0:DH, :], op0=MULT, op1=SUB,
                    )
                    da = mpool.tile([DH, QC], fp32, name="da", tag="da")
                    nc.vector.tensor_tensor(
                        out=da[:], in0=o2[32:48, :], in1=c_ob[32:48, :], op=ADD
                    )
                    d_ap = da[:]
                else:
                    nneg = None
                    d_ap = o2[32:48, :]
                y0 = mpool.tile([DH, QC], fp32, name="y0", tag="y0")
                nc.vector.tensor_scalar(
                    out=y0[:].bitcast(i32),
                    in0=d_ap.bitcast(i32),
                    scalar1=K_RECIP,
                    scalar2=-1.0,
                    op0=SUB,
                    op1=MULT,
                )
                t1 = mpool.tile([DH, QC], fp32, name="t1", tag="t1")
                nc.vector.tensor_tensor(out=t1[:], in0=d_ap, in1=y0[:], op=MULT)
                z1 = mpool.tile([DH, QC], fp32, name="z1", tag="z1")
                nc.vector.scalar_tensor_tensor(
                    out=z1[:], in0=t1[:], scalar=2.0, in1=y0[:],
                    op0=SUB, op1=MULT,
                )
                ob = mpool.tile([DH, QC], fp32, name="ob", tag="ob")
                if col_tile_ov:
                    nc.vector.tensor_tensor(
                        out=ob[:], in0=nneg[:], in1=z1[:], op=MULT
                    )
                else:
                    nc.vector.scalar_tensor_tensor(
                        out=ob[:], in0=o2[0:DH, :], scalar=-1.0, in1=z1[:],
                        op0=MULT, op1=MULT,
                    )
                nc.sync.dma_start(out=out[:, qc * QC : (qc + 1) * QC], in_=ob[:])

            def body():
                # constant regions of v2 first: no data deps, runs at t=0
                nc.vector.memset(v2_v[:, :, DH:32], 0.0)
                nc.vector.memset(v2_v[:, :, 32:48], 1.0)

                nc.sync.dma_start(out=wq_sb[:], in_=wq[:])
                nc.sync.dma_start(out=wk_sb[:], in_=wk[:])
                nc.sync.dma_start(out=wv_sb[:], in_=wv[:])
                # x in halves so projection can start on the first half
                nc.sync.dma_start(out=xb_sb[:, : N // 2], in_=xb[:, : N // 2])
                nc.sync.dma_start(out=xb_sb[:, N // 2 :], in_=xb[:, N // 2 :])

                # Warm the ACT exp table (~2.7us load) immediately at t=0:
                # seed a tiny tile with DVE so the table DMA doesn't wait for
                # the weight DMA to land first.
                warm = mpool.tile([1, 32], bf16, name="warm", tag="warm")
                nc.vector.memset(warm[:], 0.25)
                nc.scalar.activation(warm[:], warm[:], EXP, scale=SCALE)

                # projection: kt first (scores depend on it), then qt chunk 0,
                # then V (needed by the first OV), then the rest of qt
                for c0, cn in ((0, 3), (3, 3), (6, 2)):
                    proj_qk_group(kt, wk_sb, c0, cn)
                proj_qk_group(qt, wq_sb, 0, 1)
                vp = pspool.tile([D_MODEL, QC], fp32, name="vp", tag="s")
                proj_v_range(vp, 0, NKJ)
                for c0, cn in ((1, 3), (4, 3), (7, 1)):
                    proj_qk_group(qt, wq_sb, c0, cn)

                for qc in range(NQC):
                    o2_p = 128 if col_tile_ov else 48
                    o2 = popool.tile([o2_p, QC], fp32, name="o2", tag="o")
                    ov_state = [0, 0]
                    # emit each group's score matmuls BEFORE the previous
                    # group's attention@V matmuls: the in-order PE then
                    # issues the score work the exp engines need next
                    pending = None
                    for gi, (c0, cn) in enumerate(GROUPS):
                        pb = score_exp_group(qc, c0, cn, gi in dve_gis)
                        if pending is not None:
                            ov_group(o2, ov_state, *pending)
                        pending = (pb, c0, cn)
                    ov_group(o2, ov_state, *pending)
                    normalize_and_store(qc, o2)

            if loop_reps is None:
                body()
            else:
                with tc.For_i(0, loop_reps, 1):
                    body()

    if legalize:
        # note: the inserted EventSemaphores are invisible to CoreSim's race
        # detector; build with legalize=False when simulating
        _legalize_pe_waits(nc)
    return nc


def _legalize_pe_waits(nc):
    """Several HW-decoded engine instruction formats (MM, AC, ...) have a
    single sync-wait slot, but Tile occasionally attaches 2-3 waits at
    slot-reuse boundaries. Hoist the extras onto EventSemaphore instructions
    (one wait each) on the same engine queue right before the instruction —
    the same mechanism as a standalone wait_ge."""
    import concourse.mybir as mybir

    skip = {"EventSemaphore", "Call"}
    n = 0
    for blk in nc.m.functions[0].blocks:
        insts = blk.instructions
        out = []
        changed = False
        for inst in insts:
            si = getattr(inst, "sync_info", None)
            if (
                inst.opcode not in skip
                and si is not None
                and si.on_wait
                and len(si.on_wait) > 1
            ):
                waits = list(si.on_wait)
                for w in waits[:-1]:
                    ev = mybir.InstEventSemaphore(
                        name=f"hoistwait_{inst.name}_{n}", ins=[], outs=[]
                    )
                    n += 1
                    ev.engine = inst.engine
                    ev.sync_info = mybir.SyncInfo(on_wait=[w], on_update=[])
                    out.append(ev)
                si.on_wait = [waits[-1]]
                changed = True
            out.append(inst)
        if changed:
            blk.instructions = out
    return nc


def _get_nc():
    if "nc" not in _NC_CACHE:
        _NC_CACHE["nc"] = _build_nc()
    return _NC_CACHE["nc"]


def make_in_maps(x, W_qkv):
    """Host-side sharding: per-head input maps for the 8 cores."""
    bf16 = ml_dtypes.bfloat16
    x = np.asarray(x, dtype=np.float32).reshape(D_MODEL, N)
    W = np.asarray(W_qkv, dtype=np.float32)
    xb = np.ascontiguousarray(x.astype(bf16))
    in_maps = []
    for h in range(NH):
        Wq = W[48 * h : 48 * h + 16]
        Wk = W[48 * h + 16 : 48 * h + 32]
        Wv = W[48 * h + 32 : 48 * h + 48]
        wq_rep = np.zeros((D_MODEL, 128), dtype=bf16)
        wk_rep = np.zeros((D_MODEL, 128), dtype=bf16)
        for i in range(3):
            wq_rep[:, 32 * i : 32 * i + 16] = Wq.T.astype(bf16)
            wk_rep[:, 32 * i : 32 * i + 16] = Wk.T.astype(bf16)
        in_maps.append(
            {
                "xb": xb,
                "wq": wq_rep,
                "wk": wk_rep,
                "wv": np.ascontiguousarray(Wv.T.astype(bf16)),
            }
        )
    return in_maps


def run_spmd(x, W_qkv, **kwargs):
    """Compile + run on 8 cores; returns BassKernelResults."""
    from concourse.bass_utils import run_bass_kernel_spmd

    nc = _get_nc()
    in_maps = make_in_maps(x, W_qkv)
    return run_bass_kernel_spmd(nc, in_maps, core_ids=list(range(NH)), **kwargs)


def kernel(x, W_qkv):
    res = run_spmd(x, W_qkv)
    outs = [res.results[h]["out"] for h in range(NH)]  # each (16, 4096) fp32
    full = np.concatenate(outs, axis=0)  # (128, 4096)
    return np.ascontiguousarray(full.reshape(1, D_MODEL, 64, 64), dtype=np.float32)
